# revision 9
# baseline (speedup 1.0000x reference)
"""Trainium2 Bass kernel for fused LoRA linear with per-sequence adapter routing.

Problem (hardcoded shapes):
  x [8192, 4096] fp32, base_weight [4096, 4096], a_cache/b_cache [512, 4096],
  16 sequences x 512 tokens, 8 adapters (rank <= 64), out [8192, 4096]:
      out = x @ base_weight.T + scaling[a(t)] * (x @ A[a(t)].T masked) @ B[a(t)]

Sharding: data-parallel over tokens. Core c handles sequences {2c, 2c+1}
(tokens [1024c, 1024c+1024)) and computes the full 4096 output features for
its tokens. Host-side prep gathers/masks/scales the per-sequence LoRA weights
(tiny) and transposes x/base_weight; all heavy matmuls run on device.

Matmul dtype: bf16 (1 cycle/row on the PE, half the HBM traffic of fp32).
fp32 PSUM accumulation; output staged to HBM as bf16 and upcast on the host.
Rel error ~2.6e-3, well inside the 2e-2 gate.

Schedule (per core): the xa prologue (lora A @ x, psum banks 6/7) is
interleaved with chunk 0's base matmuls on banks 0-5 while the 32 xT k-tiles
stream in, so the PE never idles waiting for the full x load. Each n-chunk's
accumulation group is closed by the lora matmul; PSUM drain is split across
the vector and scalar engines, and out stores go through the scalar engine's
DGE ring so W prefetch (sync ring, one chunk ahead) never stalls on drains.
"""
import numpy as np
import ml_dtypes

import concourse.bass as bass
import concourse.mybir as mybir
from concourse.bass_utils import run_bass_kernel_spmd

P = 128
NCORES = 8
T_CORE = 1024            # tokens per core (2 sequences)
K = 4096                 # in features
N = 4096                 # out features
KT = K // P              # 32 k-tiles
NCHUNK = 512             # psum free dim per matmul
NC_N = N // NCHUNK       # 8 n-chunks
TT = T_CORE // P         # 8 t-tiles per core
SEQ_LEN = 512
MAX_RANK = 64
WRING = KT               # W ring: one full n-chunk (32 slots)

F32 = mybir.dt.float32
BF16 = mybir.dt.bfloat16
NPBF16 = ml_dtypes.bfloat16

_PROGRAM = None  # cached (nc,) build


def _build_program():
    nc = bass.Bass()
    xT_d = nc.dram_tensor("xT", [K, T_CORE], BF16, kind="ExternalInput")
    wt_d = nc.dram_tensor("wt", [K, N], BF16, kind="ExternalInput")
    at_d = nc.dram_tensor("at", [P, KT * P], BF16, kind="ExternalInput")
    bs_d = nc.dram_tensor("bs", [P, N], BF16, kind="ExternalInput")
    out_d = nc.dram_tensor("out", [T_CORE, N], BF16, kind="ExternalOutput")

    from contextlib import ExitStack
    with ExitStack() as ctx:
        e = ctx.enter_context
        xT_s = e(nc.sbuf_tensor("xT_s", [P, KT * T_CORE], BF16))      # 64 KB/part
        wt_s = e(nc.sbuf_tensor("wt_s", [P, WRING * NCHUNK], BF16))   # 32 KB/part
        at_s = e(nc.sbuf_tensor("at_s", [P, KT * P], BF16))           # 8 KB/part
        bs_s = e(nc.sbuf_tensor("bs_s", [P, N], BF16))                # 8 KB/part
        xaT_s = e(nc.sbuf_tensor("xaT_s", [P, T_CORE], BF16))         # 2 KB/part
        os_s = e(nc.sbuf_tensor("os_s", [P, TT * NCHUNK], BF16))      # 8 KB/part
        banks = [e(nc.psum_tensor(f"pbank{i}", [P, NCHUNK], F32)) for i in range(8)]
        s_at = e(nc.semaphore("s_at"))
        s_bs = e(nc.semaphore("s_bs"))
        w_sems = [e(nc.semaphore(f"s_w{i}")) for i in range(WRING)]
        xt_sems = [e(nc.semaphore(f"s_xt{i}")) for i in range(KT)]
        s_wfree = e(nc.semaphore("s_wfree"))
        s_pexa = e(nc.semaphore("s_pexa"))
        s_xa = e(nc.semaphore("s_xa"))
        s_cpv = e(nc.semaphore("s_cpv"))     # vector drains (even banks)
        s_cps = e(nc.semaphore("s_cps"))     # scalar drains (odd banks)
        s_bank = e(nc.semaphore("s_bank"))
        od_sems = [e(nc.semaphore(f"s_od{i}")) for i in range(TT)]
        block = e(nc.Block(no_gpsimd_drain=True))

        def drain_wait(tensor_or_other, c, j):
            """Wait until bank j's chunk-c drain copy has completed."""
            if j % 2 == 0:
                tensor_or_other.wait_ge(s_cpv, c * 4 + j // 2 + 1)
            else:
                tensor_or_other.wait_ge(s_cps, c * 4 + (j - 1) // 2 + 1)

        @block.sync
        def _(sync):
            # lora A as one contiguous load, then interleave xT tiles with
            # chunk-0 W tiles so the xa prologue and chunk 0 stream together.
            sync.dma_start(out=at_s[:], in_=at_d[:]).then_inc(s_at, 16)
            for k in range(KT):
                sync.dma_start(
                    out=xT_s[:, k * T_CORE:(k + 1) * T_CORE],
                    in_=xT_d[k * P:(k + 1) * P, :],
                ).then_inc(xt_sems[k], 16)
                sync.dma_start(
                    out=wt_s[:, k * NCHUNK:(k + 1) * NCHUNK],
                    in_=wt_d[k * P:(k + 1) * P, 0:NCHUNK],
                ).then_inc(w_sems[k], 16)
            sync.dma_start(out=bs_s[:], in_=bs_d[:]).then_inc(s_bs, 16)

            # W stream, one chunk ahead of compute.
            for c in range(1, NC_N):
                for k in range(KT):
                    sync.wait_ge(s_wfree, (c - 1) * KT + k + 1)
                    sync.dma_start(
                        out=wt_s[:, k * NCHUNK:(k + 1) * NCHUNK],
                        in_=wt_d[k * P:(k + 1) * P, c * NCHUNK:(c + 1) * NCHUNK],
                    ).then_inc(w_sems[k], 16)

        @block.tensor
        def _(tensor):
            # ---- streamed prologue: xa (banks 6/7) + chunk0 j=0..5 ----
            # xaT_full[r, t]; seq0 valid rows 0:64 (t 0:512), seq1 rows
            # 64:128 (t 512:1024).
            tensor.wait_ge(s_at, 16)
            for k in range(KT):
                tensor.wait_ge(xt_sems[k], 16)
                a_sl = at_s[:, k * P:(k + 1) * P]
                m0 = tensor.matmul(
                    banks[6][:], lhsT=a_sl,
                    rhs=xT_s[:, k * T_CORE: k * T_CORE + SEQ_LEN],
                    start=(k == 0), stop=(k == KT - 1))
                m1 = tensor.matmul(
                    banks[7][:], lhsT=a_sl,
                    rhs=xT_s[:, k * T_CORE + SEQ_LEN:(k + 1) * T_CORE],
                    start=(k == 0), stop=(k == KT - 1))
                tensor.wait_ge(w_sems[k], 16)
                w_sl = wt_s[:, k * NCHUNK:(k + 1) * NCHUNK]
                for j in range(6):
                    tensor.matmul(
                        banks[j][:],
                        lhsT=xT_s[:, k * T_CORE + j * P: k * T_CORE + (j + 1) * P],
                        rhs=w_sl,
                        start=(k == 0), stop=False)
            m0.then_inc(s_pexa, 1)
            m1.then_inc(s_pexa, 1)

            # chunk0 lora closes for j=0..5 (j<=3 only needs bank6's drain)
            tensor.wait_ge(s_bs, 16)
            tensor.wait_ge(s_xa, 1)
            for j in range(4):
                tensor.matmul(
                    banks[j][:],
                    lhsT=xaT_s[:, j * P:(j + 1) * P],
                    rhs=bs_s[:, 0:NCHUNK],
                    start=False, stop=True).then_inc(s_bank, 1)
            tensor.wait_ge(s_xa, 2)
            for j in (4, 5):
                tensor.matmul(
                    banks[j][:],
                    lhsT=xaT_s[:, j * P:(j + 1) * P],
                    rhs=bs_s[:, 0:NCHUNK],
                    start=False, stop=True).then_inc(s_bank, 1)
            # chunk0 j=6,7 base + lora (banks 6/7 drained by the xa copies)
            for k in range(KT):
                w_sl = wt_s[:, k * NCHUNK:(k + 1) * NCHUNK]
                for j in (6, 7):
                    mm = tensor.matmul(
                        banks[j][:],
                        lhsT=xT_s[:, k * T_CORE + j * P: k * T_CORE + (j + 1) * P],
                        rhs=w_sl,
                        start=(k == 0), stop=False)
                mm.then_inc(s_wfree, 1)  # release W(c0, k)
            for j in (6, 7):
                tensor.matmul(
                    banks[j][:],
                    lhsT=xaT_s[:, j * P:(j + 1) * P],
                    rhs=bs_s[:, 0:NCHUNK],
                    start=False, stop=True).then_inc(s_bank, 1)

            # ---- chunks 1..7 ----
            # Hoisted drain waits before each chunk keep the k-loop free of
            # per-matmul semaphore checks (they expose LDWEIGHTS otherwise);
            # the k=31 pass interleaves each bank's closing lora matmul so
            # drains start ~2us before the chunk boundary.
            for c in range(1, NC_N):
                tensor.wait_ge(s_cpv, (c - 1) * 4 + 4)
                tensor.wait_ge(s_cps, (c - 1) * 4 + 4)
                for k in range(KT):
                    tensor.wait_ge(w_sems[k], 16 * (c + 1))
                    w_sl = wt_s[:, k * NCHUNK:(k + 1) * NCHUNK]
                    for j in range(TT):
                        mm = tensor.matmul(
                            banks[j][:],
                            lhsT=xT_s[:, k * T_CORE + j * P: k * T_CORE + (j + 1) * P],
                            rhs=w_sl,
                            start=(k == 0), stop=False)
                        if k == KT - 1:
                            tensor.matmul(
                                banks[j][:],
                                lhsT=xaT_s[:, j * P:(j + 1) * P],
                                rhs=bs_s[:, c * NCHUNK:(c + 1) * NCHUNK],
                                start=False, stop=True).then_inc(s_bank, 1)
                    mm.then_inc(s_wfree, 1)  # j=7 done implies j=0..6 (pc order)

        @block.vector
        def _(vector):
            # zero xaT, then copy the valid xa quadrants in
            vector.memset(xaT_s[:], 0.0)
            vector.wait_ge(s_pexa, 1)
            vector.tensor_copy(xaT_s[0:MAX_RANK, 0:SEQ_LEN],
                               banks[6][0:MAX_RANK, :]).then_inc(s_xa, 1)
            vector.wait_ge(s_pexa, 2)
            vector.tensor_copy(xaT_s[MAX_RANK:P, SEQ_LEN:T_CORE],
                               banks[7][MAX_RANK:P, :]).then_inc(s_xa, 1)
            # even-bank drains: psum fp32 -> bf16 staging
            for c in range(NC_N):
                for j in (0, 2, 4, 6):
                    vector.wait_ge(s_bank, c * TT + j + 1)
                    if c >= 1:
                        vector.wait_ge(od_sems[j], 16 * c)
                    vector.tensor_copy(os_s[:, j * NCHUNK:(j + 1) * NCHUNK],
                                        banks[j][:]).then_inc(s_cpv, 1)

        @block.scalar
        def _(scalar):
            # odd-bank drains + all out stores (own DGE ring)
            for c in range(NC_N):
                for j in (1, 3, 5, 7):
                    scalar.wait_ge(s_bank, c * TT + j + 1)
                    if c >= 1:
                        scalar.wait_ge(od_sems[j], 16 * c)
                    scalar.copy(os_s[:, j * NCHUNK:(j + 1) * NCHUNK],
                                banks[j][:]).then_inc(s_cps, 1)
                for j in range(TT):
                    if j % 2 == 0:
                        drain_wait(scalar, c, j)
                    scalar.dma_start(
                        out=out_d[j * P:(j + 1) * P, c * NCHUNK:(c + 1) * NCHUNK],
                        in_=os_s[:, j * NCHUNK:(j + 1) * NCHUNK],
                    ).then_inc(od_sems[j], 16)

    return nc


def _get_program():
    global _PROGRAM
    if _PROGRAM is None:
        _PROGRAM = _build_program()
    return _PROGRAM


def _host_prep(x, a_cache, b_cache, base_weight, scaling,
               q_start_loc, q_seqlens, adapter_ids, rank_offset, ranks):
    """Build the 8 per-core input maps (sharding + tiny routing gathers)."""
    x = np.asarray(x, np.float32)
    a_cache = np.asarray(a_cache, np.float32)
    b_cache = np.asarray(b_cache, np.float32)
    base_weight = np.asarray(base_weight, np.float32)
    scaling = np.asarray(scaling, np.float32)
    q_start_loc = np.asarray(q_start_loc, np.int64)
    q_seqlens = np.asarray(q_seqlens, np.int64)
    adapter_ids = np.asarray(adapter_ids, np.int64)
    rank_offset = np.asarray(rank_offset, np.int64)
    ranks = np.asarray(ranks, np.int64)

    T = x.shape[0]
    assert T == NCORES * T_CORE
    # exact reference routing: per-token adapter, then check 512-block uniformity
    tok = np.arange(T)
    seq_idx = np.searchsorted(q_start_loc, tok, side="right") - 1
    tok_adapter = adapter_ids[seq_idx]
    blocks = tok_adapter.reshape(T // SEQ_LEN, SEQ_LEN)
    assert (blocks == blocks[:, :1]).all(), "non-uniform 512-token blocks"
    block_adapter = blocks[:, 0]  # [16]

    wt = np.ascontiguousarray(base_weight.T).astype(NPBF16)  # [K, N], shared

    in_maps = []
    for c in range(NCORES):
        rows = slice(c * T_CORE, (c + 1) * T_CORE)
        xT = x[rows].T.astype(NPBF16)  # [K, T_CORE] bf16, contiguous
        a_pad = np.zeros((P, K), np.float32)   # [rank slot, K]
        bs = np.zeros((P, N), np.float32)
        for s in range(2):  # two sequences per core
            a = int(block_adapter[2 * c + s])
            r = int(ranks[a])
            idxs = rank_offset[a, :r]
            a_pad[s * MAX_RANK: s * MAX_RANK + r] = a_cache[idxs]
            bs[s * MAX_RANK: s * MAX_RANK + r, :] = b_cache[idxs] * scaling[a]
        # at[p, kt*128 + r] = a_pad[r, kt*128 + p]  (per-k-tile lhsT layout)
        at = np.ascontiguousarray(
            a_pad.T.reshape(KT, P, P).transpose(1, 0, 2).reshape(P, KT * P)
        ).astype(NPBF16)
        in_maps.append({"xT": xT, "wt": wt, "at": at,
                        "bs": bs.astype(NPBF16)})
    return in_maps


LAST_RESULT = None  # BassKernelResults of the most recent run (for profiling)


def kernel(**inputs) -> np.ndarray:
    global LAST_RESULT
    nc = _get_program()
    in_maps = _host_prep(**inputs)
    res = run_bass_kernel_spmd(nc, in_maps, core_ids=list(range(NCORES)))
    LAST_RESULT = res
    return np.concatenate(
        [res.results[c]["out"].astype(np.float32) for c in range(NCORES)], axis=0)


# revision 11
# speedup vs baseline: 1.0263x; 1.0263x over previous
"""Trainium2 Bass kernel for fused LoRA linear with per-sequence adapter routing.

Problem (hardcoded shapes):
  x [8192, 4096] fp32, base_weight [4096, 4096], a_cache/b_cache [512, 4096],
  16 sequences x 512 tokens, 8 adapters (rank <= 64), out [8192, 4096]:
      out = x @ base_weight.T + scaling[a(t)] * (x @ A[a(t)].T masked) @ B[a(t)]

Sharding: data-parallel over tokens. Core c handles sequences {2c, 2c+1}
(tokens [1024c, 1024c+1024)) and computes the full 4096 output features for
its tokens. Host-side prep gathers/masks/scales the per-sequence LoRA weights
(tiny) and transposes x/base_weight; all heavy matmuls run on device.

Matmul dtype: bf16 (1 cycle/row on the PE, half the HBM traffic of fp32).
fp32 PSUM accumulation; output staged to HBM as bf16 and upcast on the host.
Rel error ~2.6e-3, well inside the 2e-2 gate.

Schedule (per core): the xa prologue (lora A @ x, psum banks 6/7) is
interleaved with chunk 0's base matmuls on banks 0-5 while the 32 xT k-tiles
stream in, so the PE never idles waiting for the full x load. Each n-chunk's
accumulation group is closed by the lora matmul; PSUM drain is split across
the vector and scalar engines, and out stores go through the scalar engine's
DGE ring so W prefetch (sync ring, one chunk ahead) never stalls on drains.
"""
import numpy as np
import ml_dtypes

import concourse.bass as bass
import concourse.mybir as mybir
from concourse.bass_utils import run_bass_kernel_spmd

P = 128
NCORES = 8
T_CORE = 1024            # tokens per core (2 sequences)
K = 4096                 # in features
N = 4096                 # out features
KT = K // P              # 32 k-tiles
NCHUNK = 512             # psum free dim per matmul
NC_N = N // NCHUNK       # 8 n-chunks
TT = T_CORE // P         # 8 t-tiles per core
SEQ_LEN = 512
MAX_RANK = 64
WRING = KT               # W ring: one full n-chunk (32 slots)

F32 = mybir.dt.float32
BF16 = mybir.dt.bfloat16
NPBF16 = ml_dtypes.bfloat16

_PROGRAM = None  # cached (nc,) build


def _build_program():
    nc = bass.Bass()
    xT_d = nc.dram_tensor("xT", [K, T_CORE], BF16, kind="ExternalInput")
    wt_d = nc.dram_tensor("wt", [K, N], BF16, kind="ExternalInput")
    at_d = nc.dram_tensor("at", [P, KT * P], BF16, kind="ExternalInput")
    bs_d = nc.dram_tensor("bs", [P, N], BF16, kind="ExternalInput")
    out_d = nc.dram_tensor("out", [T_CORE, N], BF16, kind="ExternalOutput")

    from contextlib import ExitStack
    with ExitStack() as ctx:
        e = ctx.enter_context
        xT_s = e(nc.sbuf_tensor("xT_s", [P, KT * T_CORE], BF16))      # 64 KB/part
        wt_s = e(nc.sbuf_tensor("wt_s", [P, WRING * NCHUNK], BF16))   # 32 KB/part
        at_s = e(nc.sbuf_tensor("at_s", [P, KT * P], BF16))           # 8 KB/part
        bs_s = e(nc.sbuf_tensor("bs_s", [P, N], BF16))                # 8 KB/part
        xaT_s = e(nc.sbuf_tensor("xaT_s", [P, T_CORE], BF16))         # 2 KB/part
        os_s = e(nc.sbuf_tensor("os_s", [P, TT * NCHUNK], BF16))      # 8 KB/part
        banks = [e(nc.psum_tensor(f"pbank{i}", [P, NCHUNK], F32)) for i in range(8)]
        s_at = e(nc.semaphore("s_at"))
        s_bs = e(nc.semaphore("s_bs"))
        w_sems = [e(nc.semaphore(f"s_w{i}")) for i in range(WRING)]
        xt_sems = [e(nc.semaphore(f"s_xt{i}")) for i in range(KT)]
        s_wfree = e(nc.semaphore("s_wfree"))
        s_pexa = e(nc.semaphore("s_pexa"))
        s_xa = e(nc.semaphore("s_xa"))
        s_cpv = e(nc.semaphore("s_cpv"))     # vector drains (even banks)
        s_cps = e(nc.semaphore("s_cps"))     # scalar drains (odd banks)
        s_bank = e(nc.semaphore("s_bank"))
        od_sems = [e(nc.semaphore(f"s_od{i}")) for i in range(TT)]
        block = e(nc.Block(no_gpsimd_drain=True))

        def drain_wait(tensor_or_other, c, j):
            """Wait until bank j's chunk-c drain copy has completed."""
            if j % 2 == 0:
                tensor_or_other.wait_ge(s_cpv, c * 4 + j // 2 + 1)
            else:
                tensor_or_other.wait_ge(s_cps, c * 4 + (j - 1) // 2 + 1)

        @block.sync
        def _(sync):
            # lora A as one contiguous load, then interleave xT tiles with
            # chunk-0 W tiles so the xa prologue and chunk 0 stream together.
            sync.dma_start(out=at_s[:], in_=at_d[:]).then_inc(s_at, 16)
            for k in range(KT):
                sync.dma_start(
                    out=xT_s[:, k * T_CORE:(k + 1) * T_CORE],
                    in_=xT_d[k * P:(k + 1) * P, :],
                ).then_inc(xt_sems[k], 16)
                sync.dma_start(
                    out=wt_s[:, k * NCHUNK:(k + 1) * NCHUNK],
                    in_=wt_d[k * P:(k + 1) * P, 0:NCHUNK],
                ).then_inc(w_sems[k], 16)
            sync.dma_start(out=bs_s[:], in_=bs_d[:]).then_inc(s_bs, 16)

            # W stream, one chunk ahead of compute.
            for c in range(1, NC_N):
                for k in range(KT):
                    sync.wait_ge(s_wfree, (c - 1) * KT + k + 1)
                    sync.dma_start(
                        out=wt_s[:, k * NCHUNK:(k + 1) * NCHUNK],
                        in_=wt_d[k * P:(k + 1) * P, c * NCHUNK:(c + 1) * NCHUNK],
                    ).then_inc(w_sems[k], 16)

        @block.tensor
        def _(tensor):
            # ---- streamed prologue: xa (banks 6/7) + chunk0 j=0..5 ----
            # xaT_full[r, t]; seq0 valid rows 0:64 (t 0:512), seq1 rows
            # 64:128 (t 512:1024).
            tensor.wait_ge(s_at, 16)
            for k in range(KT):
                tensor.wait_ge(xt_sems[k], 16)
                a_sl = at_s[:, k * P:(k + 1) * P]
                m0 = tensor.matmul(
                    banks[6][:], lhsT=a_sl,
                    rhs=xT_s[:, k * T_CORE: k * T_CORE + SEQ_LEN],
                    start=(k == 0), stop=(k == KT - 1))
                m1 = tensor.matmul(
                    banks[7][:], lhsT=a_sl,
                    rhs=xT_s[:, k * T_CORE + SEQ_LEN:(k + 1) * T_CORE],
                    start=(k == 0), stop=(k == KT - 1))
                tensor.wait_ge(w_sems[k], 16)
                w_sl = wt_s[:, k * NCHUNK:(k + 1) * NCHUNK]
                for j in range(6):
                    tensor.matmul(
                        banks[j][:],
                        lhsT=xT_s[:, k * T_CORE + j * P: k * T_CORE + (j + 1) * P],
                        rhs=w_sl,
                        start=(k == 0), stop=False)
            m0.then_inc(s_pexa, 1)
            m1.then_inc(s_pexa, 1)

            # chunk0 lora closes for j=0..5 (j<=3 only needs bank6's drain)
            tensor.wait_ge(s_bs, 16)
            tensor.wait_ge(s_xa, 1)
            for j in range(4):
                tensor.matmul(
                    banks[j][:],
                    lhsT=xaT_s[:, j * P:(j + 1) * P],
                    rhs=bs_s[:, 0:NCHUNK],
                    start=False, stop=True).then_inc(s_bank, 1)
            tensor.wait_ge(s_xa, 2)
            for j in (4, 5):
                tensor.matmul(
                    banks[j][:],
                    lhsT=xaT_s[:, j * P:(j + 1) * P],
                    rhs=bs_s[:, 0:NCHUNK],
                    start=False, stop=True).then_inc(s_bank, 1)
            # chunk0 j=6,7 base + lora (banks 6/7 drained by the xa copies)
            for k in range(KT):
                w_sl = wt_s[:, k * NCHUNK:(k + 1) * NCHUNK]
                for j in (6, 7):
                    mm = tensor.matmul(
                        banks[j][:],
                        lhsT=xT_s[:, k * T_CORE + j * P: k * T_CORE + (j + 1) * P],
                        rhs=w_sl,
                        start=(k == 0), stop=False)
                mm.then_inc(s_wfree, 1)  # release W(c0, k)
            for j in (6, 7):
                tensor.matmul(
                    banks[j][:],
                    lhsT=xaT_s[:, j * P:(j + 1) * P],
                    rhs=bs_s[:, 0:NCHUNK],
                    start=False, stop=True).then_inc(s_bank, 1)

            # ---- chunks 1..7, processed as two half-chunks of 4 banks ----
            # While half h computes, the other half's banks drain with a
            # ~28us window, so the hoisted drain waits below are always
            # already satisfied and the matmul stream never breaks.
            for c in range(1, NC_N):
                for h in (0, 1):
                    js = range(4 * h, 4 * h + 4)
                    for j in js:
                        drain_wait(tensor, c - 1, j)
                    for k in range(KT):
                        if h == 0:
                            tensor.wait_ge(w_sems[k], 16 * (c + 1))
                        w_sl = wt_s[:, k * NCHUNK:(k + 1) * NCHUNK]
                        for j in js:
                            mm = tensor.matmul(
                                banks[j][:],
                                lhsT=xT_s[:, k * T_CORE + j * P: k * T_CORE + (j + 1) * P],
                                rhs=w_sl,
                                start=(k == 0), stop=False)
                        if h == 1:
                            mm.then_inc(s_wfree, 1)  # slot k fully consumed
                    for j in js:
                        tensor.matmul(
                            banks[j][:],
                            lhsT=xaT_s[:, j * P:(j + 1) * P],
                            rhs=bs_s[:, c * NCHUNK:(c + 1) * NCHUNK],
                            start=False, stop=True).then_inc(s_bank, 1)

        @block.vector
        def _(vector):
            # zero xaT, then copy the valid xa quadrants in
            vector.memset(xaT_s[:], 0.0)
            vector.wait_ge(s_pexa, 1)
            vector.tensor_copy(xaT_s[0:MAX_RANK, 0:SEQ_LEN],
                               banks[6][0:MAX_RANK, :]).then_inc(s_xa, 1)
            vector.wait_ge(s_pexa, 2)
            vector.tensor_copy(xaT_s[MAX_RANK:P, SEQ_LEN:T_CORE],
                               banks[7][MAX_RANK:P, :]).then_inc(s_xa, 1)
            # even-bank drains: psum fp32 -> bf16 staging
            for c in range(NC_N):
                for j in (0, 2, 4, 6):
                    vector.wait_ge(s_bank, c * TT + j + 1)
                    if c >= 1:
                        vector.wait_ge(od_sems[j], 16 * c)
                    vector.tensor_copy(os_s[:, j * NCHUNK:(j + 1) * NCHUNK],
                                        banks[j][:]).then_inc(s_cpv, 1)

        @block.scalar
        def _(scalar):
            # odd-bank drains + all out stores (own DGE ring), per half-chunk
            for c in range(NC_N):
                for h in (0, 1):
                    for j in (4 * h + 1, 4 * h + 3):
                        scalar.wait_ge(s_bank, c * TT + j + 1)
                        if c >= 1:
                            scalar.wait_ge(od_sems[j], 16 * c)
                        scalar.copy(os_s[:, j * NCHUNK:(j + 1) * NCHUNK],
                                    banks[j][:]).then_inc(s_cps, 1)
                    for j in range(4 * h, 4 * h + 4):
                        if j % 2 == 0:
                            drain_wait(scalar, c, j)
                        scalar.dma_start(
                            out=out_d[j * P:(j + 1) * P, c * NCHUNK:(c + 1) * NCHUNK],
                            in_=os_s[:, j * NCHUNK:(j + 1) * NCHUNK],
                        ).then_inc(od_sems[j], 16)

    return nc


def _get_program():
    global _PROGRAM
    if _PROGRAM is None:
        _PROGRAM = _build_program()
    return _PROGRAM


def _host_prep(x, a_cache, b_cache, base_weight, scaling,
               q_start_loc, q_seqlens, adapter_ids, rank_offset, ranks):
    """Build the 8 per-core input maps (sharding + tiny routing gathers)."""
    x = np.asarray(x, np.float32)
    a_cache = np.asarray(a_cache, np.float32)
    b_cache = np.asarray(b_cache, np.float32)
    base_weight = np.asarray(base_weight, np.float32)
    scaling = np.asarray(scaling, np.float32)
    q_start_loc = np.asarray(q_start_loc, np.int64)
    q_seqlens = np.asarray(q_seqlens, np.int64)
    adapter_ids = np.asarray(adapter_ids, np.int64)
    rank_offset = np.asarray(rank_offset, np.int64)
    ranks = np.asarray(ranks, np.int64)

    T = x.shape[0]
    assert T == NCORES * T_CORE
    # exact reference routing: per-token adapter, then check 512-block uniformity
    tok = np.arange(T)
    seq_idx = np.searchsorted(q_start_loc, tok, side="right") - 1
    tok_adapter = adapter_ids[seq_idx]
    blocks = tok_adapter.reshape(T // SEQ_LEN, SEQ_LEN)
    assert (blocks == blocks[:, :1]).all(), "non-uniform 512-token blocks"
    block_adapter = blocks[:, 0]  # [16]

    wt = np.ascontiguousarray(base_weight.T).astype(NPBF16)  # [K, N], shared

    in_maps = []
    for c in range(NCORES):
        rows = slice(c * T_CORE, (c + 1) * T_CORE)
        xT = x[rows].T.astype(NPBF16)  # [K, T_CORE] bf16, contiguous
        a_pad = np.zeros((P, K), np.float32)   # [rank slot, K]
        bs = np.zeros((P, N), np.float32)
        for s in range(2):  # two sequences per core
            a = int(block_adapter[2 * c + s])
            r = int(ranks[a])
            idxs = rank_offset[a, :r]
            a_pad[s * MAX_RANK: s * MAX_RANK + r] = a_cache[idxs]
            bs[s * MAX_RANK: s * MAX_RANK + r, :] = b_cache[idxs] * scaling[a]
        # at[p, kt*128 + r] = a_pad[r, kt*128 + p]  (per-k-tile lhsT layout)
        at = np.ascontiguousarray(
            a_pad.T.reshape(KT, P, P).transpose(1, 0, 2).reshape(P, KT * P)
        ).astype(NPBF16)
        in_maps.append({"xT": xT, "wt": wt, "at": at,
                        "bs": bs.astype(NPBF16)})
    return in_maps


LAST_RESULT = None  # BassKernelResults of the most recent run (for profiling)


def kernel(**inputs) -> np.ndarray:
    global LAST_RESULT
    nc = _get_program()
    in_maps = _host_prep(**inputs)
    res = run_bass_kernel_spmd(nc, in_maps, core_ids=list(range(NCORES)))
    LAST_RESULT = res
    return np.concatenate(
        [res.results[c]["out"].astype(np.float32) for c in range(NCORES)], axis=0)


# revision 14
# speedup vs baseline: 1.0273x; 1.0009x over previous
"""Trainium2 Bass kernel for fused LoRA linear with per-sequence adapter routing.

Problem (hardcoded shapes):
  x [8192, 4096] fp32, base_weight [4096, 4096], a_cache/b_cache [512, 4096],
  16 sequences x 512 tokens, 8 adapters (rank <= 64), out [8192, 4096]:
      out = x @ base_weight.T + scaling[a(t)] * (x @ A[a(t)].T masked) @ B[a(t)]

Sharding: data-parallel over tokens. Core c handles sequences {2c, 2c+1}
(tokens [1024c, 1024c+1024)) and computes the full 4096 output features for
its tokens. Host-side prep gathers/masks/scales the per-sequence LoRA weights
(tiny) and transposes x/base_weight; all heavy matmuls run on device.

Matmul dtype: bf16 (1 cycle/row on the PE, half the HBM traffic of fp32).
fp32 PSUM accumulation; output staged to HBM as bf16 and upcast on the host.
Rel error ~2.6e-3, well inside the 2e-2 gate.

Schedule (per core): the xa prologue (lora A @ x, psum banks 6/7) is
interleaved with chunk 0's base matmuls on banks 0-5 while the 32 xT k-tiles
stream in, so the PE never idles waiting for the full x load. Each n-chunk's
accumulation group is closed by the lora matmul; PSUM drain is split across
the vector and scalar engines, and out stores go through the scalar engine's
DGE ring so W prefetch (sync ring, one chunk ahead) never stalls on drains.
"""
import numpy as np
import ml_dtypes

import concourse.bass as bass
import concourse.mybir as mybir
from concourse.bass_utils import run_bass_kernel_spmd

P = 128
NCORES = 8
T_CORE = 1024            # tokens per core (2 sequences)
K = 4096                 # in features
N = 4096                 # out features
KT = K // P              # 32 k-tiles
NCHUNK = 512             # psum free dim per matmul
NC_N = N // NCHUNK       # 8 n-chunks
TT = T_CORE // P         # 8 t-tiles per core
SEQ_LEN = 512
MAX_RANK = 64
WRING = KT               # W ring: one full n-chunk (32 slots)

F32 = mybir.dt.float32
BF16 = mybir.dt.bfloat16
NPBF16 = ml_dtypes.bfloat16

_PROGRAM = None  # cached (nc,) build


def _build_program():
    nc = bass.Bass()
    xT_d = nc.dram_tensor("xT", [K, T_CORE], BF16, kind="ExternalInput")
    wt_d = nc.dram_tensor("wt", [K, N], BF16, kind="ExternalInput")
    at_d = nc.dram_tensor("at", [P, KT * P], BF16, kind="ExternalInput")
    bs_d = nc.dram_tensor("bs", [P, N], BF16, kind="ExternalInput")
    out_d = nc.dram_tensor("out", [T_CORE, N], BF16, kind="ExternalOutput")

    from contextlib import ExitStack
    with ExitStack() as ctx:
        e = ctx.enter_context
        xT_s = e(nc.sbuf_tensor("xT_s", [P, KT * T_CORE], BF16))      # 64 KB/part
        wt_s = e(nc.sbuf_tensor("wt_s", [P, WRING * NCHUNK], BF16))   # 32 KB/part
        at_s = e(nc.sbuf_tensor("at_s", [P, KT * P], BF16))           # 8 KB/part
        bs_s = e(nc.sbuf_tensor("bs_s", [P, N], BF16))                # 8 KB/part
        xaT_s = e(nc.sbuf_tensor("xaT_s", [P, T_CORE], BF16))         # 2 KB/part
        os_s = e(nc.sbuf_tensor("os_s", [P, TT * NCHUNK], BF16))      # 8 KB/part
        banks = [e(nc.psum_tensor(f"pbank{i}", [P, NCHUNK], F32)) for i in range(8)]
        at_sems = [e(nc.semaphore(f"s_at{i}")) for i in range(2)]
        s_bs = e(nc.semaphore("s_bs"))
        w_sems = [e(nc.semaphore(f"s_w{i}")) for i in range(WRING)]
        xt_sems = [e(nc.semaphore(f"s_xt{i}")) for i in range(KT)]
        s_wfree = e(nc.semaphore("s_wfree"))
        s_pexa = e(nc.semaphore("s_pexa"))
        s_xa = e(nc.semaphore("s_xa"))
        s_cpv = e(nc.semaphore("s_cpv"))     # vector drains (even banks)
        s_cps = e(nc.semaphore("s_cps"))     # scalar drains (odd banks)
        s_bank = e(nc.semaphore("s_bank"))
        od_sems = [e(nc.semaphore(f"s_od{i}")) for i in range(TT)]
        block = e(nc.Block(no_gpsimd_drain=True))

        def drain_wait(tensor_or_other, c, j):
            """Wait until bank j's chunk-c drain copy has completed."""
            if j % 2 == 0:
                tensor_or_other.wait_ge(s_cpv, c * 4 + j // 2 + 1)
            else:
                tensor_or_other.wait_ge(s_cps, c * 4 + (j - 1) // 2 + 1)

        @block.sync
        def _(sync):
            # lora A in two halves (xa k=0 only needs the first), then
            # interleave xT tiles with chunk-0 W tiles so the xa prologue
            # and chunk 0 stream together.
            AH = KT * P // 2
            sync.dma_start(out=at_s[:, 0:AH],
                           in_=at_d[:, 0:AH]).then_inc(at_sems[0], 16)
            sync.dma_start(out=at_s[:, AH:],
                           in_=at_d[:, AH:]).then_inc(at_sems[1], 16)
            for k in range(KT):
                sync.dma_start(
                    out=xT_s[:, k * T_CORE:(k + 1) * T_CORE],
                    in_=xT_d[k * P:(k + 1) * P, :],
                ).then_inc(xt_sems[k], 16)
                sync.dma_start(
                    out=wt_s[:, k * NCHUNK:(k + 1) * NCHUNK],
                    in_=wt_d[k * P:(k + 1) * P, 0:NCHUNK],
                ).then_inc(w_sems[k], 16)
            sync.dma_start(out=bs_s[:], in_=bs_d[:]).then_inc(s_bs, 16)

            # W stream, one chunk ahead of compute.
            for c in range(1, NC_N):
                for k in range(KT):
                    sync.wait_ge(s_wfree, (c - 1) * KT + k + 1)
                    sync.dma_start(
                        out=wt_s[:, k * NCHUNK:(k + 1) * NCHUNK],
                        in_=wt_d[k * P:(k + 1) * P, c * NCHUNK:(c + 1) * NCHUNK],
                    ).then_inc(w_sems[k], 16)

        @block.tensor
        def _(tensor):
            # ---- streamed prologue: xa (banks 6/7) + chunk0 j=0..5 ----
            # xaT_full[r, t]; seq0 valid rows 0:64 (t 0:512), seq1 rows
            # 64:128 (t 512:1024).
            tensor.wait_ge(at_sems[0], 16)
            for k in range(KT):
                if k == KT // 2:
                    tensor.wait_ge(at_sems[1], 16)
                tensor.wait_ge(xt_sems[k], 16)
                a_sl = at_s[:, k * P:(k + 1) * P]
                m0 = tensor.matmul(
                    banks[6][:], lhsT=a_sl,
                    rhs=xT_s[:, k * T_CORE: k * T_CORE + SEQ_LEN],
                    start=(k == 0), stop=(k == KT - 1))
                m1 = tensor.matmul(
                    banks[7][:], lhsT=a_sl,
                    rhs=xT_s[:, k * T_CORE + SEQ_LEN:(k + 1) * T_CORE],
                    start=(k == 0), stop=(k == KT - 1))
                tensor.wait_ge(w_sems[k], 16)
                w_sl = wt_s[:, k * NCHUNK:(k + 1) * NCHUNK]
                for j in range(6):
                    tensor.matmul(
                        banks[j][:],
                        lhsT=xT_s[:, k * T_CORE + j * P: k * T_CORE + (j + 1) * P],
                        rhs=w_sl,
                        start=(k == 0), stop=False)
            m0.then_inc(s_pexa, 1)
            m1.then_inc(s_pexa, 1)

            # chunk0 lora closes for j=0..5 (j<=3 only needs bank6's drain)
            tensor.wait_ge(s_bs, 16)
            tensor.wait_ge(s_xa, 1)
            for j in range(4):
                tensor.matmul(
                    banks[j][:],
                    lhsT=xaT_s[:, j * P:(j + 1) * P],
                    rhs=bs_s[:, 0:NCHUNK],
                    start=False, stop=True).then_inc(s_bank, 1)
            tensor.wait_ge(s_xa, 2)
            for j in (4, 5):
                tensor.matmul(
                    banks[j][:],
                    lhsT=xaT_s[:, j * P:(j + 1) * P],
                    rhs=bs_s[:, 0:NCHUNK],
                    start=False, stop=True).then_inc(s_bank, 1)
            # chunk0 j=6,7 base + lora (banks 6/7 drained by the xa copies)
            for k in range(KT):
                w_sl = wt_s[:, k * NCHUNK:(k + 1) * NCHUNK]
                for j in (6, 7):
                    mm = tensor.matmul(
                        banks[j][:],
                        lhsT=xT_s[:, k * T_CORE + j * P: k * T_CORE + (j + 1) * P],
                        rhs=w_sl,
                        start=(k == 0), stop=False)
                mm.then_inc(s_wfree, 1)  # release W(c0, k)
            for j in (6, 7):
                tensor.matmul(
                    banks[j][:],
                    lhsT=xaT_s[:, j * P:(j + 1) * P],
                    rhs=bs_s[:, 0:NCHUNK],
                    start=False, stop=True).then_inc(s_bank, 1)

            # ---- chunks 1..7, processed as two half-chunks of 4 banks ----
            # While half h computes, the other half's banks drain with a
            # ~28us window, so the hoisted drain waits below are always
            # already satisfied and the matmul stream never breaks.
            for c in range(1, NC_N):
                for h in (0, 1):
                    js = range(4 * h, 4 * h + 4)
                    for j in js:
                        drain_wait(tensor, c - 1, j)
                    for k in range(KT):
                        if h == 0:
                            tensor.wait_ge(w_sems[k], 16 * (c + 1))
                        w_sl = wt_s[:, k * NCHUNK:(k + 1) * NCHUNK]
                        for j in js:
                            mm = tensor.matmul(
                                banks[j][:],
                                lhsT=xT_s[:, k * T_CORE + j * P: k * T_CORE + (j + 1) * P],
                                rhs=w_sl,
                                start=(k == 0), stop=False)
                        if h == 1:
                            mm.then_inc(s_wfree, 1)  # slot k fully consumed
                    for j in js:
                        tensor.matmul(
                            banks[j][:],
                            lhsT=xaT_s[:, j * P:(j + 1) * P],
                            rhs=bs_s[:, c * NCHUNK:(c + 1) * NCHUNK],
                            start=False, stop=True).then_inc(s_bank, 1)

        @block.vector
        def _(vector):
            # zero xaT, then copy the valid xa quadrants in
            vector.memset(xaT_s[:], 0.0)
            vector.wait_ge(s_pexa, 1)
            vector.tensor_copy(xaT_s[0:MAX_RANK, 0:SEQ_LEN],
                               banks[6][0:MAX_RANK, :]).then_inc(s_xa, 1)
            vector.wait_ge(s_pexa, 2)
            vector.tensor_copy(xaT_s[MAX_RANK:P, SEQ_LEN:T_CORE],
                               banks[7][MAX_RANK:P, :]).then_inc(s_xa, 1)
            # even-bank drains: psum fp32 -> bf16 staging
            for c in range(NC_N):
                for j in (0, 2, 4, 6):
                    vector.wait_ge(s_bank, c * TT + j + 1)
                    if c >= 1:
                        vector.wait_ge(od_sems[j], 16 * c)
                    vector.tensor_copy(os_s[:, j * NCHUNK:(j + 1) * NCHUNK],
                                        banks[j][:]).then_inc(s_cpv, 1)

        @block.scalar
        def _(scalar):
            # odd-bank drains + all out stores (own DGE ring), per half-chunk
            for c in range(NC_N):
                for h in (0, 1):
                    for j in (4 * h + 1, 4 * h + 3):
                        scalar.wait_ge(s_bank, c * TT + j + 1)
                        if c >= 1:
                            scalar.wait_ge(od_sems[j], 16 * c)
                        scalar.copy(os_s[:, j * NCHUNK:(j + 1) * NCHUNK],
                                    banks[j][:]).then_inc(s_cps, 1)
                    for j in range(4 * h, 4 * h + 4):
                        if j % 2 == 0:
                            drain_wait(scalar, c, j)
                        scalar.dma_start(
                            out=out_d[j * P:(j + 1) * P, c * NCHUNK:(c + 1) * NCHUNK],
                            in_=os_s[:, j * NCHUNK:(j + 1) * NCHUNK],
                        ).then_inc(od_sems[j], 16)

    return nc


def _get_program():
    global _PROGRAM
    if _PROGRAM is None:
        _PROGRAM = _build_program()
    return _PROGRAM


def _host_prep(x, a_cache, b_cache, base_weight, scaling,
               q_start_loc, q_seqlens, adapter_ids, rank_offset, ranks):
    """Build the 8 per-core input maps (sharding + tiny routing gathers)."""
    x = np.asarray(x, np.float32)
    a_cache = np.asarray(a_cache, np.float32)
    b_cache = np.asarray(b_cache, np.float32)
    base_weight = np.asarray(base_weight, np.float32)
    scaling = np.asarray(scaling, np.float32)
    q_start_loc = np.asarray(q_start_loc, np.int64)
    q_seqlens = np.asarray(q_seqlens, np.int64)
    adapter_ids = np.asarray(adapter_ids, np.int64)
    rank_offset = np.asarray(rank_offset, np.int64)
    ranks = np.asarray(ranks, np.int64)

    T = x.shape[0]
    assert T == NCORES * T_CORE
    # exact reference routing: per-token adapter, then check 512-block uniformity
    tok = np.arange(T)
    seq_idx = np.searchsorted(q_start_loc, tok, side="right") - 1
    tok_adapter = adapter_ids[seq_idx]
    blocks = tok_adapter.reshape(T // SEQ_LEN, SEQ_LEN)
    assert (blocks == blocks[:, :1]).all(), "non-uniform 512-token blocks"
    block_adapter = blocks[:, 0]  # [16]

    wt = np.ascontiguousarray(base_weight.T).astype(NPBF16)  # [K, N], shared

    in_maps = []
    for c in range(NCORES):
        rows = slice(c * T_CORE, (c + 1) * T_CORE)
        xT = x[rows].T.astype(NPBF16)  # [K, T_CORE] bf16, contiguous
        a_pad = np.zeros((P, K), np.float32)   # [rank slot, K]
        bs = np.zeros((P, N), np.float32)
        for s in range(2):  # two sequences per core
            a = int(block_adapter[2 * c + s])
            r = int(ranks[a])
            idxs = rank_offset[a, :r]
            a_pad[s * MAX_RANK: s * MAX_RANK + r] = a_cache[idxs]
            bs[s * MAX_RANK: s * MAX_RANK + r, :] = b_cache[idxs] * scaling[a]
        # at[p, kt*128 + r] = a_pad[r, kt*128 + p]  (per-k-tile lhsT layout)
        at = np.ascontiguousarray(
            a_pad.T.reshape(KT, P, P).transpose(1, 0, 2).reshape(P, KT * P)
        ).astype(NPBF16)
        in_maps.append({"xT": xT, "wt": wt, "at": at,
                        "bs": bs.astype(NPBF16)})
    return in_maps


LAST_RESULT = None  # BassKernelResults of the most recent run (for profiling)


def kernel(**inputs) -> np.ndarray:
    global LAST_RESULT
    nc = _get_program()
    in_maps = _host_prep(**inputs)
    res = run_bass_kernel_spmd(nc, in_maps, core_ids=list(range(NCORES)))
    LAST_RESULT = res
    return np.concatenate(
        [res.results[c]["out"].astype(np.float32) for c in range(NCORES)], axis=0)
